# revision 18
# baseline (speedup 1.0000x reference)
"""Bahdanau-style attention kernel for Trainium2, data-parallel over batch on 8 cores.

Math (per batch row b):
    h_proj = hidden @ Wh.T + b_attn                      [128]
    energy[s, :] = tanh(h_proj + embs[s] @ We.T)         [S, 128]
    att[s] = v . energy[s, :]                            [S]
    out = softmax_S(where(mask==0, -1e10, att))

Device strategy per core (8 batch rows, S=4096):
  - Host preps layout only: batch shard, fp16 cast, transpose of seq_embs
    to [b, e, s] so DMA streams at full rate and PE gets the contraction
    dim (e) on partitions directly.
  - DMA: consts + b0 quarters via SWDGE (gpsimd ring, earliest start);
    b1-7 full-row 1MiB transfers on the sync HWDGE ring. Both rings feed
    the same 16 SDMA engines, so descriptor generation overlaps.
  - PE: We-matmuls [e,128]x[e,512] -> e_projT in PSUM tiles [128,1536];
    one-hot-column v-matmuls contract d and scatter each (b, 512-chunk)
    att row into a persistent [64, 512] PSUM accumulator (partition =
    8*b + s//512), pre-seeded with the mask bias via an identity matmul.
  - ACT: tanh with per-partition bias h_projT[:, b] in 1536-wide calls
    (amortizes the ~352-cycle per-call overhead); later one 512-wide exp
    with accum_out for the softmax row-sums.
  - Softmax skips max-subtraction: |att| <= ||v||_1 ~ 5.7 so exp is safe;
    mask bias of -30 matches where(mask==0,-1e10) to float precision.
"""

import numpy as np

B = 64
S = 4096
D = 128  # dec_dim == emb_dim == 128
NCORES = 8
BPC = B // NCORES  # 8 batch rows per core
NP = 64  # att accumulator partitions: 8*b + s//512
FW = 512  # att accumulator free width (one PSUM bank)
CPB = S // FW  # 8 512-chunks per batch row

_COMPILED = {}


def _build_bass():
    import concourse.bacc as bacc
    import concourse.mybir as mybir
    from concourse.tile import TileContext

    f32 = mybir.dt.float32
    fp16 = mybir.dt.float16
    AF = mybir.ActivationFunctionType

    nc = bacc.Bacc(
        "TRN2", target_bir_lowering=False, debug=False, num_devices=NCORES
    )

    embsT = nc.dram_tensor("embsT", [BPC, D, S], fp16, kind="ExternalInput")
    # fp16 consts: mask64 [64,512] | I64 [64,64] | WeT [128,128] | vstrip [128,128]
    FPW = FW + NP + D + 2 * NP
    fppack = nc.dram_tensor("fppack", [D, FPW], fp16, kind="ExternalInput")
    # f32 consts: WhT [128,128] | hiddenT [128,8] | b_attn [128,1]
    #             qsel64 [64,8] | qsel64T [8,64]
    CPW = D + BPC + 1 + BPC + NP
    cpack = nc.dram_tensor("cpack", [D, CPW], f32, kind="ExternalInput")
    out_d = nc.dram_tensor("out", [NP, FW], f32, kind="ExternalOutput")

    TW = 1536  # e_proj / tanh tile width (3 PSUM banks)

    with TileContext(nc) as tc:
        with (
            tc.tile_pool(name="consts", bufs=1) as consts,
            tc.tile_pool(name="embs0", bufs=1) as embs0_pool,
            tc.tile_pool(name="embsab", bufs=2) as embsab_pool,
            tc.tile_pool(name="embs16", bufs=5) as embs16_pool,
            tc.tile_pool(name="energy", bufs=7) as energy_pool,
            tc.tile_pool(name="post", bufs=1) as post,
            tc.tile_pool(name="ps_big", bufs=2, space="PSUM") as ps_big,
            tc.tile_pool(name="ps_att", bufs=1, space="PSUM") as ps_att,
        ):
            # All input DMAs ride one HWDGE ring in strict priority order:
            # consts -> b0 quarters -> b1-7 bulk. A second ring would let
            # the bulk steal engine bandwidth from the latency-critical
            # head (engines round-robin between rings at packet level).
            fppack_sb = consts.tile([D, FPW], fp16)
            nc.sync.dma_start(out=fppack_sb, in_=fppack[:, :])
            cpack_sb = consts.tile([D, CPW], f32)
            nc.sync.dma_start(out=cpack_sb, in_=cpack[:, :])
            o = 0
            mask64_sb = fppack_sb[0:NP, o : o + FW]; o += FW
            id64_sb = fppack_sb[0:NP, o : o + NP]; o += NP
            WeT_sb = fppack_sb[:, o : o + D]; o += D
            vstrip_sb = fppack_sb[:, o : o + 2 * NP]
            o = 0
            WhT_sb = cpack_sb[:, o : o + D]; o += D
            hiddenT_sb = cpack_sb[:, o : o + BPC]; o += BPC
            b_attn_sb = cpack_sb[:, o : o + 1]; o += 1
            qsel_sb = cpack_sb[0:NP, o : o + BPC]; o += BPC
            qselT_sb = cpack_sb[0:BPC, o : o + NP]

            # b0 arrives as a small first chunk (fast first tanh) + the rest
            QW = 1024
            et0a = embs0_pool.tile([D, QW], fp16, tag="et0a")
            nc.sync.dma_start(out=et0a, in_=embsT[0, :, 0:QW])
            et0b = embs0_pool.tile([D, S - QW], fp16, tag="et0b")
            nc.sync.dma_start(out=et0b, in_=embsT[0, :, QW:S])
            # b1/b2 in two pieces (finer consumer dependencies while the
            # stream ramps), b3-7 as full-row 1MiB transfers
            ets = [None]
            for b in range(1, BPC):
                if b <= 2:
                    eta = embsab_pool.tile([D, TW], fp16, tag="eta")
                    nc.sync.dma_start(out=eta, in_=embsT[b, :, 0:TW])
                    etb = embsab_pool.tile([D, S - TW], fp16, tag="etb")
                    nc.sync.dma_start(out=etb, in_=embsT[b, :, TW:S])
                    ets.append((eta, etb))
                else:
                    et = embs16_pool.tile([D, S], fp16, tag="et")
                    nc.sync.dma_start(out=et, in_=embsT[b, :, :])
                    ets.append(et)

            # Pre-load the ACT exp/tanh table set on a dummy so the ~1.3us
            # ACT_TABLE_LOAD runs during the DMA warm-up, off critical path.
            dummy = consts.tile([1, 8], f32)
            nc.vector.memset(dummy[:, :], 0.0)
            dummy2 = consts.tile([1, 8], f32)
            nc.scalar.activation(out=dummy2[:, :], in_=dummy[:, :], func=AF.Tanh)

            # h_projT[d, b] = sum_k WhT[k, d] * hiddenT[k, b] + b_attn[d]
            hp_ps = ps_big.tile([D, BPC], f32, tag="ps")
            nc.tensor.matmul(hp_ps[:, :], WhT_sb[:, :], hiddenT_sb[:, :])
            hprojT_sb = consts.tile([D, BPC], f32)
            nc.vector.tensor_scalar_add(hprojT_sb[:, :], hp_ps[:, :], b_attn_sb[:, 0:1])

            # att accumulator [64, 512]: partition 8*b + s//512, free s%512.
            # Seeded with the mask bias (0/-30) via I64 @ mask64 so the
            # epilogue's exp reads att_ps directly.
            att_ps = ps_att.tile([NP, FW], f32)
            n_mm_tot = 1 + BPC * CPB
            n_vmm = 0

            def emit_mask_mm():
                nonlocal n_vmm
                nc.tensor.matmul(
                    att_ps[:, :],
                    id64_sb[:, :],
                    mask64_sb[:, :],
                    start=True,
                    stop=False,
                    skip_group_check=True,
                )
                n_vmm += 1

            def emit_vmms(pending):
                nonlocal n_vmm
                for en_t, b, off, w in pending:
                    for m in range(w // FW):
                        p = CPB * b + (off + m * FW) // FW
                        nc.tensor.matmul(
                            att_ps[:, :],
                            vstrip_sb[:, NP - p : 2 * NP - p],
                            en_t[:, m * FW : (m + 1) * FW],
                            start=False,
                            stop=(n_vmm >= n_mm_tot - 1),
                            skip_group_check=True,
                        )
                        n_vmm += 1

            def do_tile(b, off, w, src, src_off):
                """e_proj chunk [128, w] at s-offset off for batch b, reading
                src[:, src_off:src_off+w]; returns the tanh'd energy tile."""
                pe_t = ps_big.tile([D, TW], f32, tag="ps")
                for m in range(w // FW):
                    nc.tensor.matmul(
                        pe_t[:, m * FW : (m + 1) * FW],
                        WeT_sb[:, :],
                        src[:, src_off + m * FW : src_off + (m + 1) * FW],
                    )
                en_t = energy_pool.tile([D, TW], fp16)
                nc.scalar.activation(
                    out=en_t[:, 0:w],
                    in_=pe_t[:, 0:w],
                    func=AF.Tanh,
                    bias=hprojT_sb[:, b : b + 1],
                    scale=1.0,
                )
                return (en_t, b, off, w)

            prev = []
            for b in range(BPC):
                cur = []
                if b == 0:
                    emit_mask_mm()
                    cur.append(do_tile(0, 0, QW, et0a, 0))
                    cur.append(do_tile(0, QW, TW, et0b, 0))
                    cur.append(do_tile(0, QW + TW, TW, et0b, TW))
                elif b <= 2:
                    eta, etb = ets[b]
                    cur.append(do_tile(b, 0, TW, eta, 0))
                    cur.append(do_tile(b, TW, TW, etb, 0))
                    cur.append(do_tile(b, 2 * TW, S - 2 * TW, etb, TW))
                else:
                    for off, w in [(0, TW), (TW, TW), (2 * TW, S - 2 * TW)]:
                        cur.append(do_tile(b, off, w, ets[b], off))
                emit_vmms(prev)
                prev = cur
            emit_vmms(prev)

            # softmax over s (per batch row): p = exp(att); accum_out gives
            # per-partition (512-chunk) partial sums in the same ACT pass.
            p_sb = post.tile([NP, FW], f32)
            partials_sb = post.tile([NP, 1], f32)
            nc.scalar.activation(
                out=p_sb[:, :],
                in_=att_ps[:, :],
                func=AF.Exp,
                accum_out=partials_sb[:, 0:1],
            )
            # denom[b] = sum over the 8 chunk-partials of batch b
            den_ps = ps_big.tile([BPC, 1], f32, tag="ps")
            nc.tensor.matmul(den_ps[:, :], qsel_sb[:, :], partials_sb[:, 0:1])
            recip8_sb = post.tile([BPC, 1], f32)
            nc.vector.reciprocal(recip8_sb[:, :], den_ps[:, :])
            # spread 1/denom back to the 64-partition layout
            r64_ps = ps_big.tile([NP, 1], f32, tag="ps")
            nc.tensor.matmul(r64_ps[:, :], qselT_sb[:, :], recip8_sb[:, 0:1])
            recip64_sb = post.tile([NP, 1], f32)
            nc.vector.tensor_copy(recip64_sb[:, :], r64_ps[:, :])

            out_sb = post.tile([NP, FW], f32)
            nc.vector.tensor_scalar_mul(out_sb[:, :], p_sb[:, :], recip64_sb[:, 0:1])
            nc.sync.dma_start(out=out_d[:, :], in_=out_sb[:, :])

    nc.compile()
    return nc


def _get_nc():
    if "nc" not in _COMPILED:
        _COMPILED["nc"] = _build_bass()
    return _COMPILED["nc"]


def _prep_inputs(hidden, seq_embs, mask, W_attn, b_attn, v_w):
    """Host-side prep: shard over batch, fp16 cast + relayouts (no math)."""
    hidden = np.asarray(hidden, dtype=np.float32)
    seq_embs = np.asarray(seq_embs, dtype=np.float32)
    mask = np.asarray(mask)
    W_attn = np.asarray(W_attn, dtype=np.float32)
    b_attn = np.asarray(b_attn, dtype=np.float32)
    v_w = np.asarray(v_w, dtype=np.float32)

    WhT = np.ascontiguousarray(W_attn[:, :D].T)
    WeT = np.ascontiguousarray(W_attn[:, D:].T.astype(np.float16))
    b_col = np.ascontiguousarray(b_attn.reshape(D, 1))
    vstrip = np.zeros((D, 2 * NP), dtype=np.float16)
    vstrip[:, NP] = v_w[0].astype(np.float16)
    qsel = np.zeros((NP, BPC), dtype=np.float32)
    for p in range(NP):
        qsel[p, p // CPB] = 1.0
    qselT = np.ascontiguousarray(qsel.T)

    FPW = FW + NP + D + 2 * NP
    CPW = D + BPC + 1 + BPC + NP

    in_maps = []
    for c in range(NCORES):
        bsl = slice(c * BPC, (c + 1) * BPC)
        embsT = np.ascontiguousarray(
            np.transpose(seq_embs[:, bsl, :], (1, 2, 0)).astype(np.float16)
        )  # [8, 128, 4096] fp16
        mask64 = (
            (mask[bsl].astype(np.float32).reshape(NP, FW) - 1.0) * 30.0
        ).astype(np.float16)
        fppack = np.zeros((D, FPW), dtype=np.float16)
        o = 0
        fppack[:NP, o : o + FW] = mask64; o += FW
        fppack[:NP, o : o + NP] = np.eye(NP, dtype=np.float16); o += NP
        fppack[:, o : o + D] = WeT; o += D
        fppack[:, o : o + 2 * NP] = vstrip

        hiddenT = np.ascontiguousarray(hidden[bsl].T)  # [128, 8]
        cpack = np.zeros((D, CPW), dtype=np.float32)
        o = 0
        cpack[:, o : o + D] = WhT; o += D
        cpack[:, o : o + BPC] = hiddenT; o += BPC
        cpack[:, o : o + 1] = b_col; o += 1
        cpack[:NP, o : o + BPC] = qsel; o += BPC
        cpack[:BPC, o : o + NP] = qselT
        in_maps.append(
            {
                "embsT": embsT,
                "fppack": fppack,
                "cpack": cpack,
            }
        )
    return in_maps


def kernel(hidden, seq_embs, mask, W_attn, b_attn, v_w, **run_kwargs):
    from concourse.bass_utils import run_bass_kernel_spmd

    nc = _get_nc()
    in_maps = _prep_inputs(hidden, seq_embs, mask, W_attn, b_attn, v_w)
    res = run_bass_kernel_spmd(
        nc, in_maps, core_ids=list(range(NCORES)), **run_kwargs
    )
    out = np.concatenate(
        [r["out"].reshape(BPC, S) for r in res.results], axis=0
    ).astype(np.float32)
    if run_kwargs:
        kernel.last_results = res  # stash for the profiling harness
    return out
